# revision 16
# baseline (speedup 1.0000x reference)
"""TRN2 Bass/Tile kernel for nn_DotProductAttention (softmax over the QUERY axis).

reference:
    scores  = einsum('bqd,bkd->bqk', q, k) / sqrt(64)
    weights = softmax(scores, axis=1)          # over q, NOT k!
    out     = einsum('bqk,bkd->bqd', weights, v)

Transposed-score formulation: T = K @ Q^T (shape [k, q]) puts the softmax
axis (q) on the free dim, so the normalizer Z[k] is a free-axis row sum and
1/Z folds into V (Vs = V/Z) ahead of the second matmul.

v2 structure (single fused main loop):
  - The AV accumulation is interleaved chunk-by-chunk with the score/exp
    pipeline, so the PE stream is dense (p-state stays at full clock) and
    there is no separate second phase.
  - exp work is split across engines: most chunks use the ACT engine's exact
    Exp; ~1/3 use a Schraudolph bit-trick exp on the DVE (one tensor_scalar
    producing int16 = round(s*A + B), reinterpreted as bf16).  The softmax
    renormalization cancels the approximation's mean error; measured e2e
    rel-err ~9e-3 against the 2e-2 gate.
  - Z row sums ride a fused (E0+E1)+accum op on GPSIMD (most chunks) or DVE.
  - PSUM: 2 x [128,1024] score tiles (4 banks) rotate, [128,2048] f32 O^T
    accumulator (4 banks) lives for the whole loop.

Sharding: B=16 batches, data-parallel over 8 cores => 2 batches per core,
(b,d)-packed into the two 64-partition halves of [128,*] tiles.
"""

import math
from contextlib import ExitStack

import numpy as np

import concourse.bass as bass  # noqa: F401
import concourse.mybir as mybir
import concourse.tile as tile
from bass_rust import add_dep_helper
from concourse import bacc, bass_utils
from concourse.masks import make_identity

FP32 = mybir.dt.float32
BF16 = mybir.dt.bfloat16
I16 = mybir.dt.int16

N_CORES = 8
B_FULL = 16
BPC = B_FULL // N_CORES  # batches per core = 2
S = 2048
D = 64
NCH = S // 128  # 16 key chunks of 128
SCALE = 1.0 / math.sqrt(D)

# Schraudolph bf16 exp: bf16 = bitcast(int16(round(s_raw * A_SCH + B_SCH)))
# approximates exp(s_raw / 8).  A = 2^7/ln2/8; B centers the bf16 exponent.
A_SCH = float(2.0**7 / math.log(2.0) / 8.0)
B_SCH = 16255.0

# chunk order index o = 2*i + b.  First 6 chunks (head warmup) always ACT.
# 13 chunks on the DVE schraudolph path, the rest exact exp on ACT.
SCHRAUD = frozenset({7, 9, 11, 12, 14, 17, 19, 21, 22, 24, 26, 27, 29})
# Schraudolph chunks whose Z-fold first two levels ride gpsimd (fp adds are
# legal on Pool for SBUF operands); the remaining few use a full-width DVE op.
Z_GP = frozenset({9, 11, 12, 14, 17, 19, 21, 22, 24, 26})


def emit_kernel(ctx: ExitStack, tc, q, k, v, o):
    """Emit the per-core Tile program. q/k/v/o are DRAM APs of [BPC, S, D] f32."""
    nc = tc.nc

    const_pool = ctx.enter_context(tc.tile_pool(name="const", bufs=1))
    big = ctx.enter_context(tc.tile_pool(name="big", bufs=1))
    dram = ctx.enter_context(tc.tile_pool(name="dram", bufs=1, space="DRAM"))
    # PSUM: 2 rotating [128,1024] f32 score tiles (4 banks) + the transpose
    # scratch tiles, and a separate 4-bank [128,2048] f32 O^T accumulator.
    ps = ctx.enter_context(tc.tile_pool(name="ps", bufs=2, space="PSUM"))
    pspot = ctx.enter_context(tc.tile_pool(name="pspot", bufs=1, space="PSUM"))

    identb = const_pool.tile([128, 128], BF16, name="identb")
    make_identity(nc, identb)
    zw = const_pool.tile([128, 128], BF16, name="zw")
    nc.vector.memset(zw[:], 0.0)

    # (b,d)-packed transposed operands: partitions 0:64 = batch0 d, 64:128 = b1 d.
    QT = big.tile([128, S], BF16, name="QT")
    KT = big.tile([128, S], BF16, name="KT")
    # staging for Q/K chunks in (m, b, d) column layout, s on partitions
    qstage = big.tile([128, S], FP32, name="qstage")
    kstage = big.tile([128, S], FP32, name="kstage")
    qbf = big.tile([128, S], BF16, name="qbf")
    kbf = big.tile([128, S], BF16, name="kbf")
    kbf_dram = dram.tile([S, 128], BF16, name="kbf_dram")
    qbf_dram = dram.tile([S // 2, 128], BF16, name="qbf_dram")
    # V chunks [128 k, 64 d] f32 and Vs = V / Z (bf16)
    V = big.tile([128, BPC * NCH * D], FP32, name="V")
    Vs = big.tile([128, BPC * NCH * D], BF16, name="Vs")
    # per-chunk stats columns: [zh0, zh1, z, 1/z]
    stats = big.tile([128, BPC * NCH * 4], FP32, name="stats")
    # E[(b*NCH+i)*S :+ S] = exp(scores/8): [128 k, 2048 q] bf16, fully resident
    E = big.tile([128, BPC * NCH * S], BF16, name="E")
    # scratch sinks for the Z folds (one per engine so no cross-engine WAW)
    zscrV = big.tile([128, 1024], BF16, name="zscrV")
    zG1 = big.tile([128, 1024], BF16, name="zG1")
    zG2 = big.tile([128, 512], BF16, name="zG2")
    # O^T staging ((b,d) packed on partitions, q on free) and O natural layout
    OT = big.tile([128, S], BF16, name="OT")
    O_all = big.tile([128, S], FP32, name="O_all")

    # ---------------- phase A: loads, casts, transposes ----------------
    QRT = NCH // 4  # chunks per quarter-DMA
    load_order = [("q", 0), ("k", 0), ("q", 1), ("k", 1), ("q", 2), ("q", 3),
                  ("k", 2), ("k", 3)]
    qdma = {}
    for t, Q in load_order:
        src, stg = (q, qstage) if t == "q" else (k, kstage)
        ssl = slice(Q * QRT * 128, (Q + 1) * QRT * 128)
        for b in range(BPC):
            dma = nc.sync.dma_start(
                stg[:, ssl].rearrange("p (m b d) -> p m b d", m=QRT, b=BPC, d=D)[
                    :, :, b, :
                ],
                src[b, ssl, :].rearrange("(m p) d -> p m d", p=128),
            )
            qdma[(t, Q, b)] = dma
    # V loads on the scalar queue (HWDGE), held behind the first k quarter so
    # they do not steal HBM bandwidth from the critical q/k head loads.
    for b in range(BPC):
        vdma = nc.scalar.dma_start(
            V[:].rearrange("p (b m d) -> p b m d", b=BPC, m=NCH)[:, b, :, :],
            v[b].rearrange("(m p) d -> p m d", p=128),
        )
        add_dep_helper(
            vdma.ins, qdma[("k", 1, BPC - 1)].ins, sync=True,
            reason="delay V behind head loads",
        )

    # casts f32 -> bf16, quarter-granular in load-arrival order.  The q casts
    # ride the otherwise-idle gpsimd engine (CAST is Pool-legal for SBUF);
    # K is prescaled by A_SCH on DVE so psum scores arrive as A_SCH * s_raw:
    # the Schraudolph op is then a single-op ADD (the two-op mult+add form
    # with int16 output crashes the DVE) and the ACT exp uses a smaller scale.
    for t, Q in load_order:
        stg, bft = (qstage, qbf) if t == "q" else (kstage, kbf)
        csl = slice(Q * QRT * 128, (Q + 1) * QRT * 128)
        if t == "q":
            nc.gpsimd.tensor_copy(bft[:, csl], stg[:, csl])
        else:
            nc.vector.tensor_scalar_mul(bft[:, csl], stg[:, csl], A_SCH)

    # PE transposes for the chunks needed before the DMA roundtrips land:
    # q0..3, k0..3, q4..7.  Copies alternate ACT/DVE.
    for idx, (t, m) in enumerate(
        [("q", 0), ("q", 1), ("q", 2), ("q", 3),
         ("k", 0), ("k", 1), ("k", 2), ("k", 3),
         ("q", 4), ("q", 5), ("q", 6), ("q", 7)]
    ):
        bft, dst = (qbf, QT) if t == "q" else (kbf, KT)
        pt = ps.tile([128, 128], BF16, tag="ps", name=f"pt_{t}{m}")
        nc.tensor.transpose(pt[:], bft[:, m * 128 : (m + 1) * 128], identb[:])
        if idx % 2 == 0:
            nc.scalar.copy(dst[:, m * 128 : (m + 1) * 128], pt[:])
        else:
            nc.vector.tensor_copy(dst[:, m * 128 : (m + 1) * 128], pt[:])

    # DMA roundtrip (bf16) transposes for KT chunks 4..15 and QT chunks 8..15
    nc.sync.dma_start(
        kbf_dram[512:S, :].rearrange("(m p) c -> p m c", p=128),
        kbf[:, 512:S].rearrange("p (m c) -> p m c", m=NCH - 4),
    )
    nc.sync.dma_start(
        qbf_dram[:].rearrange("(m p) c -> p m c", p=128),
        qbf[:, 1024:S].rearrange("p (m c) -> p m c", m=NCH - 8),
    )
    nc.sync.dma_start_transpose(out=KT[:, 512:S], in_=kbf_dram[512:S, :])
    nc.sync.dma_start_transpose(out=QT[:, 1024:S], in_=qbf_dram[:])

    # ---------------- fused main loop ----------------
    pot = pspot.tile([128, S], FP32, name="pot")
    zmm = []

    def emit_zmm():
        # open the O^T accumulator: full-128-partition zeroing matmuls set
        # has_written for each bank region so the strip-sliced AV matmuls can
        # accumulate with start=False.
        for j in range(4):
            zmm.append(
                nc.tensor.matmul(
                    pot[:, j * 512 : (j + 1) * 512],
                    lhsT=zw[:],
                    rhs=QT[:, 0:512],
                    start=True,
                    stop=False,
                    skip_group_check=True,
                )
            )

    def emit_unit(i, b, H):
        """scores (2 matmuls into a [128,1024] psum tile) + exp for q-half H."""
        cid = b * NCH + i
        o_idx = 2 * i + b
        sct = ps.tile([128, 1024], FP32, tag="ps", name=f"sc{i}_{b}_{H}")
        for jj in range(2):
            base = H * 1024 + jj * 512
            nc.tensor.matmul(
                sct[:, jj * 512 : (jj + 1) * 512],
                lhsT=KT[b * 64 : (b + 1) * 64, i * 128 : (i + 1) * 128],
                rhs=QT[b * 64 : (b + 1) * 64, base : base + 512],
                start=True,
                stop=True,
            )
        eb = cid * S + H * 1024
        if o_idx in SCHRAUD:
            nc.vector.tensor_scalar(
                E[:, eb : eb + 1024].bitcast(I16),
                sct[:],
                B_SCH,
                None,
                mybir.AluOpType.add,
            )
        else:
            # exact exp on ACT; the per-half accumulator read gives Zh free
            nc.scalar.activation(
                E[:, eb : eb + 1024],
                sct[:],
                mybir.ActivationFunctionType.Exp,
                scale=SCALE / A_SCH,
                accum_out=stats[:, cid * 4 + H : cid * 4 + H + 1],
            )

    def emit_zfold(i, b):
        """Z = sum_q E[k,q], then 1/Z and Vs = V/Z."""
        cid = b * NCH + i
        o_idx = 2 * i + b
        sb = cid * 4
        e0 = E[:, cid * S : cid * S + 1024]
        e1 = E[:, cid * S + 1024 : cid * S + 2048]
        if o_idx not in SCHRAUD:
            # Z = Zh0 + Zh1 from the two ACT accumulator reads
            nc.vector.tensor_tensor(
                stats[:, sb + 2 : sb + 3], stats[:, sb : sb + 1],
                stats[:, sb + 1 : sb + 2], mybir.AluOpType.add,
            )
        elif o_idx in Z_GP:
            # two fold levels on gpsimd, short fused reduce on DVE
            nc.gpsimd.tensor_add(zG1[:], e0, e1)
            nc.gpsimd.tensor_add(zG2[:], zG1[:, 0:512], zG1[:, 512:1024])
            nc.vector.scalar_tensor_tensor(
                zscrV[:, 0:256], zG2[:, 0:256], 1.0, zG2[:, 256:512],
                mybir.AluOpType.mult, mybir.AluOpType.add,
                accum_out=stats[:, sb + 2 : sb + 3],
            )
        else:
            nc.vector.scalar_tensor_tensor(
                zscrV[:], e0, 1.0, e1,
                mybir.AluOpType.mult, mybir.AluOpType.add,
                accum_out=stats[:, sb + 2 : sb + 3],
            )
        nc.vector.reciprocal(stats[:, sb + 3 : sb + 4], stats[:, sb + 2 : sb + 3])
        vb = cid * D
        nc.vector.tensor_scalar_mul(
            Vs[:, vb : vb + D], V[:, vb : vb + D], stats[:, sb + 3 : sb + 4]
        )

    drainers = []

    def emit_av(i, last=False):
        """O^T[(b,d), q] += Vs_i^T @ E_i; b0/b1 in disjoint PE column strips."""
        for j in range(4):
            for b in range(BPC):
                cid = b * NCH + i
                vb = cid * D
                eb = cid * S
                mm = nc.tensor.matmul(
                    pot[b * 64 : (b + 1) * 64, j * 512 : (j + 1) * 512],
                    lhsT=Vs[:, vb : vb + D],
                    rhs=E[:, eb + j * 512 : eb + (j + 1) * 512],
                    start=False,
                    stop=(last and b == BPC - 1),
                    skip_group_check=True,
                )
                if not zmm_linked[j]:
                    add_dep_helper(
                        mm.ins, zmm[j].ins, sync=False,
                        reason="AV accumulation after bank-opening zero matmul",
                    )
            if last:
                drainers.append(j)
                emit_drain(j)

    def emit_drain(j):
        """Unpack pot q-block j, transpose to natural layout, store."""
        if j % 2 == 0:
            nc.vector.tensor_copy(
                OT[:, j * 512 : (j + 1) * 512], pot[:, j * 512 : (j + 1) * 512]
            )
        else:
            nc.scalar.copy(
                OT[:, j * 512 : (j + 1) * 512], pot[:, j * 512 : (j + 1) * 512]
            )
        o_view = O_all[:].rearrange("p (m b d) -> p m b d", m=NCH, b=BPC, d=D)
        for m in range(4 * j, 4 * j + 4):
            ptc = ps.tile([128, 128], BF16, tag="ps", name=f"ptc_{m}")
            nc.tensor.transpose(ptc[:], OT[:, m * 128 : (m + 1) * 128], identb[:])
            if m % 2 == 0:
                nc.vector.tensor_copy(O_all[:, m * 128 : (m + 1) * 128], ptc[:])
            else:
                nc.scalar.copy(O_all[:, m * 128 : (m + 1) * 128], ptc[:])
        for b in range(BPC):
            eng = nc.sync if b == 0 else nc.gpsimd
            eng.dma_start(
                o[b, 4 * j * 128 : (4 * j + 4) * 128, :].rearrange(
                    "(m p) d -> p m d", p=128
                ),
                o_view[:, 4 * j : 4 * j + 4, b, :],
            )

    zmm_linked = [False, False, False, False]

    # -- schedule --
    # i=0..2: H0 only (QT half 1 arrives via DMA roundtrip a bit later)
    for i in range(3):
        for b in range(BPC):
            emit_unit(i, b, 0)
    # i=3: first full chunk; open the accumulator banks in-stream
    for b in range(BPC):
        emit_unit(3, b, 0)
        emit_unit(3, b, 1)
        emit_zfold(3, b)
    emit_zmm()
    for b in range(BPC):
        emit_unit(4, b, 0)
        emit_unit(4, b, 1)
        emit_zfold(4, b)
    emit_av(3)
    zmm_linked = [True] * 4

    # steady state: scores/exp for chunk i, AV for chunk i-1; weave the
    # deferred H1 halves of chunks 0..2 (and their AVs) into i=5..10.
    stash_units = [(si, b) for si in range(3) for b in range(BPC)]
    for i in range(5, NCH):
        for b in range(BPC):
            emit_unit(i, b, 0)
            emit_unit(i, b, 1)
            emit_zfold(i, b)
        emit_av(i - 1)
        if i in (5, 7, 9):
            si = (i - 5) // 2
            for b in range(BPC):
                emit_unit(si, b, 1)
                emit_zfold(si, b)
        elif i in (6, 8, 10):
            emit_av((i - 6) // 2)
    emit_av(NCH - 1, last=True)


_CACHE: dict = {}


def build_program():
    if "nc" in _CACHE:
        return _CACHE["nc"]
    nc = bacc.Bacc("TRN2", target_bir_lowering=False, debug=False)
    q = nc.dram_tensor("q", [BPC, S, D], FP32, kind="ExternalInput").ap()
    k = nc.dram_tensor("k", [BPC, S, D], FP32, kind="ExternalInput").ap()
    v = nc.dram_tensor("v", [BPC, S, D], FP32, kind="ExternalInput").ap()
    o = nc.dram_tensor("o", [BPC, S, D], FP32, kind="ExternalOutput").ap()
    with tile.TileContext(nc) as tc:
        with ExitStack() as ctx:
            emit_kernel(ctx, tc, q, k, v, o)
    nc.compile()
    _CACHE["nc"] = nc
    return nc


def make_in_maps(q, k, v):
    q = np.ascontiguousarray(q, dtype=np.float32)
    k = np.ascontiguousarray(k, dtype=np.float32)
    v = np.ascontiguousarray(v, dtype=np.float32)
    assert q.shape == (B_FULL, S, D), q.shape
    return [
        {
            "q": np.ascontiguousarray(q[c * BPC : (c + 1) * BPC]),
            "k": np.ascontiguousarray(k[c * BPC : (c + 1) * BPC]),
            "v": np.ascontiguousarray(v[c * BPC : (c + 1) * BPC]),
        }
        for c in range(N_CORES)
    ]


def kernel(q, k, v, _trace=False):
    nc = build_program()
    in_maps = make_in_maps(q, k, v)
    try:
        res = bass_utils.run_bass_kernel_spmd(
            nc, in_maps, core_ids=list(range(N_CORES)), trace=_trace
        )
    except Exception:
        # This box occasionally throws a transient device error
        # (NRT_EXEC_UNIT_UNRECOVERABLE); a short pause + one retry has
        # recovered it every time it was observed.
        import time

        time.sleep(5)
        res = bass_utils.run_bass_kernel_spmd(
            nc, in_maps, core_ids=list(range(N_CORES)), trace=_trace
        )
    out = np.concatenate([r["o"] for r in res.results], axis=0)
    if _trace:
        return out, res
    return out


# revision 20
# speedup vs baseline: 1.0280x; 1.0280x over previous
"""TRN2 Bass/Tile kernel for nn_DotProductAttention (softmax over the QUERY axis).

reference:
    scores  = einsum('bqd,bkd->bqk', q, k) / sqrt(64)
    weights = softmax(scores, axis=1)          # over q, NOT k!
    out     = einsum('bqk,bkd->bqd', weights, v)

Transposed-score formulation: T = K @ Q^T (shape [k, q]) puts the softmax
axis (q) on the free dim, so the normalizer Z[k] is a free-axis row sum and
1/Z folds into V (Vs = V/Z) ahead of the second matmul.

v2 structure (single fused main loop):
  - The AV accumulation is interleaved chunk-by-chunk with the score/exp
    pipeline, so the PE stream is dense (p-state stays at full clock) and
    there is no separate second phase.
  - exp work is split across engines: most chunks use the ACT engine's exact
    Exp; ~1/3 use a Schraudolph bit-trick exp on the DVE (one tensor_scalar
    producing int16 = round(s*A + B), reinterpreted as bf16).  The softmax
    renormalization cancels the approximation's mean error; measured e2e
    rel-err ~9e-3 against the 2e-2 gate.
  - Z row sums ride a fused (E0+E1)+accum op on GPSIMD (most chunks) or DVE.
  - PSUM: 2 x [128,1024] score tiles (4 banks) rotate, [128,2048] f32 O^T
    accumulator (4 banks) lives for the whole loop.

Sharding: B=16 batches, data-parallel over 8 cores => 2 batches per core,
(b,d)-packed into the two 64-partition halves of [128,*] tiles.
"""

import math
from contextlib import ExitStack

import numpy as np

import concourse.bass as bass  # noqa: F401
import concourse.mybir as mybir
import concourse.tile as tile
from bass_rust import add_dep_helper
from concourse import bacc, bass_utils
from concourse.masks import make_identity

FP32 = mybir.dt.float32
BF16 = mybir.dt.bfloat16
I16 = mybir.dt.int16

N_CORES = 8
B_FULL = 16
BPC = B_FULL // N_CORES  # batches per core = 2
S = 2048
D = 64
NCH = S // 128  # 16 key chunks of 128
SCALE = 1.0 / math.sqrt(D)

# Schraudolph bf16 exp: bf16 = bitcast(int16(round(s_raw * A_SCH + B_SCH)))
# approximates exp(s_raw / 8).  A = 2^7/ln2/8; B centers the bf16 exponent.
A_SCH = float(2.0**7 / math.log(2.0) / 8.0)
B_SCH = 16255.0

# chunk order index o = 2*i + b.  First 6 chunks (head warmup) always ACT.
# 11 chunks on the DVE schraudolph path, the rest exact exp on ACT.
SCHRAUD = frozenset({7, 9, 12, 14, 17, 19, 22, 24, 26, 27, 29})


def emit_kernel(ctx: ExitStack, tc, q, k, v, o):
    """Emit the per-core Tile program. q/k/v/o are DRAM APs of [BPC, S, D] f32."""
    nc = tc.nc

    const_pool = ctx.enter_context(tc.tile_pool(name="const", bufs=1))
    big = ctx.enter_context(tc.tile_pool(name="big", bufs=1))
    dram = ctx.enter_context(tc.tile_pool(name="dram", bufs=1, space="DRAM"))
    # PSUM: 2 rotating [128,1024] f32 score tiles (4 banks) + the transpose
    # scratch tiles, and a separate 4-bank [128,2048] f32 O^T accumulator.
    ps = ctx.enter_context(tc.tile_pool(name="ps", bufs=2, space="PSUM"))
    pspot = ctx.enter_context(tc.tile_pool(name="pspot", bufs=1, space="PSUM"))

    identb = const_pool.tile([128, 128], BF16, name="identb")
    make_identity(nc, identb)
    zw = const_pool.tile([128, 128], BF16, name="zw")
    nc.vector.memset(zw[:], 0.0)

    # (b,d)-packed transposed operands: partitions 0:64 = batch0 d, 64:128 = b1 d.
    QT = big.tile([128, S], BF16, name="QT")
    KT = big.tile([128, S], BF16, name="KT")
    # staging for Q/K chunks in (m, b, d) column layout, s on partitions
    qstage = big.tile([128, S], FP32, name="qstage")
    kstage = big.tile([128, S], FP32, name="kstage")
    qbf = big.tile([128, S], BF16, name="qbf")
    kbf = big.tile([128, S], BF16, name="kbf")
    kbf_dram = dram.tile([S, 128], BF16, name="kbf_dram")
    qbf_dram = dram.tile([S // 2, 128], BF16, name="qbf_dram")
    # V chunks [128 k, 64 d] f32 and Vs = V / Z (bf16)
    V = big.tile([128, BPC * NCH * D], FP32, name="V")
    Vs = big.tile([128, BPC * NCH * D], BF16, name="Vs")
    # per-chunk stats columns: [zh0, zh1, z, 1/z]
    stats = big.tile([128, BPC * NCH * 4], FP32, name="stats")
    # E[(b*NCH+i)*S :+ S] = exp(scores/8): [128 k, 2048 q] bf16, fully resident
    E = big.tile([128, BPC * NCH * S], BF16, name="E")
    # scratch sinks for the Z folds (one per engine so no cross-engine WAW)
    zscrV = big.tile([128, 1024], BF16, name="zscrV")
    zG1 = big.tile([128, 1024], BF16, name="zG1")
    zG2 = big.tile([128, 512], BF16, name="zG2")
    # O^T staging ((b,d) packed on partitions, q on free) and O natural layout
    OT = big.tile([128, S], BF16, name="OT")
    O_all = big.tile([128, S], FP32, name="O_all")

    # ---------------- phase A: loads, casts, transposes ----------------
    QRT = NCH // 4  # chunks per quarter-DMA
    load_order = [("q", 0), ("k", 0), ("q", 1), ("k", 1), ("q", 2), ("q", 3),
                  ("k", 2), ("k", 3)]
    qdma = {}
    for t, Q in load_order:
        src, stg = (q, qstage) if t == "q" else (k, kstage)
        ssl = slice(Q * QRT * 128, (Q + 1) * QRT * 128)
        for b in range(BPC):
            dma = nc.sync.dma_start(
                stg[:, ssl].rearrange("p (m b d) -> p m b d", m=QRT, b=BPC, d=D)[
                    :, :, b, :
                ],
                src[b, ssl, :].rearrange("(m p) d -> p m d", p=128),
            )
            qdma[(t, Q, b)] = dma
    # V loads on the scalar queue (HWDGE), held behind the first k quarter so
    # they do not steal HBM bandwidth from the critical q/k head loads.
    for b in range(BPC):
        vdma = nc.scalar.dma_start(
            V[:].rearrange("p (b m d) -> p b m d", b=BPC, m=NCH)[:, b, :, :],
            v[b].rearrange("(m p) d -> p m d", p=128),
        )
        add_dep_helper(
            vdma.ins, qdma[("k", 1, BPC - 1)].ins, sync=True,
            reason="delay V behind head loads",
        )

    # casts f32 -> bf16, quarter-granular in load-arrival order.  The q casts
    # ride the otherwise-idle gpsimd engine (CAST is Pool-legal for SBUF);
    # K is prescaled by A_SCH on DVE so psum scores arrive as A_SCH * s_raw:
    # the Schraudolph op is then a single-op ADD (the two-op mult+add form
    # with int16 output crashes the DVE) and the ACT exp uses a smaller scale.
    for t, Q in load_order:
        stg, bft = (qstage, qbf) if t == "q" else (kstage, kbf)
        csl = slice(Q * QRT * 128, (Q + 1) * QRT * 128)
        if t == "q":
            nc.vector.tensor_copy(bft[:, csl], stg[:, csl])
        else:
            nc.vector.tensor_scalar_mul(bft[:, csl], stg[:, csl], A_SCH)

    # PE transposes for the chunks needed before the DMA roundtrips land:
    # q0..3, k0..3, q4..7.  Copies alternate ACT/DVE.
    for idx, (t, m) in enumerate(
        [("q", 0), ("q", 1), ("q", 2), ("q", 3),
         ("k", 0), ("k", 1), ("k", 2), ("k", 3),
         ("q", 4), ("q", 5), ("q", 6), ("q", 7)]
    ):
        bft, dst = (qbf, QT) if t == "q" else (kbf, KT)
        pt = ps.tile([128, 128], BF16, tag="ps", name=f"pt_{t}{m}")
        nc.tensor.transpose(pt[:], bft[:, m * 128 : (m + 1) * 128], identb[:])
        if idx % 2 == 0:
            nc.scalar.copy(dst[:, m * 128 : (m + 1) * 128], pt[:])
        else:
            nc.vector.tensor_copy(dst[:, m * 128 : (m + 1) * 128], pt[:])

    # DMA roundtrip (bf16) transposes for KT chunks 4..15 and QT chunks 8..15
    nc.sync.dma_start(
        kbf_dram[512:S, :].rearrange("(m p) c -> p m c", p=128),
        kbf[:, 512:S].rearrange("p (m c) -> p m c", m=NCH - 4),
    )
    nc.sync.dma_start(
        qbf_dram[:].rearrange("(m p) c -> p m c", p=128),
        qbf[:, 1024:S].rearrange("p (m c) -> p m c", m=NCH - 8),
    )
    nc.sync.dma_start_transpose(out=KT[:, 512:S], in_=kbf_dram[512:S, :])
    nc.sync.dma_start_transpose(out=QT[:, 1024:S], in_=qbf_dram[:])

    # ---------------- fused main loop ----------------
    pot = pspot.tile([128, S], FP32, name="pot")
    zmm = []

    def emit_zmm():
        # open the O^T accumulator: full-128-partition zeroing matmuls set
        # has_written for each bank region so the strip-sliced AV matmuls can
        # accumulate with start=False.
        for j in range(4):
            zmm.append(
                nc.tensor.matmul(
                    pot[:, j * 512 : (j + 1) * 512],
                    lhsT=zw[:],
                    rhs=QT[:, 0:512],
                    start=True,
                    stop=False,
                    skip_group_check=True,
                )
            )

    def emit_unit(i, b, H):
        """scores (2 matmuls into a [128,1024] psum tile) + exp for q-half H."""
        cid = b * NCH + i
        o_idx = 2 * i + b
        sct = ps.tile([128, 1024], FP32, tag="ps", name=f"sc{i}_{b}_{H}")
        for jj in range(2):
            base = H * 1024 + jj * 512
            nc.tensor.matmul(
                sct[:, jj * 512 : (jj + 1) * 512],
                lhsT=KT[b * 64 : (b + 1) * 64, i * 128 : (i + 1) * 128],
                rhs=QT[b * 64 : (b + 1) * 64, base : base + 512],
                start=True,
                stop=True,
            )
        eb = cid * S + H * 1024
        if o_idx in SCHRAUD:
            nc.vector.tensor_scalar(
                E[:, eb : eb + 1024].bitcast(I16),
                sct[:],
                B_SCH,
                None,
                mybir.AluOpType.add,
            )
        else:
            # exact exp on ACT; the per-half accumulator read gives Zh free
            nc.scalar.activation(
                E[:, eb : eb + 1024],
                sct[:],
                mybir.ActivationFunctionType.Exp,
                scale=SCALE / A_SCH,
                accum_out=stats[:, cid * 4 + H : cid * 4 + H + 1],
            )

    def emit_zfold(i, b):
        """Z = sum_q E[k,q], then 1/Z and Vs = V/Z."""
        cid = b * NCH + i
        o_idx = 2 * i + b
        sb = cid * 4
        e0 = E[:, cid * S : cid * S + 1024]
        e1 = E[:, cid * S + 1024 : cid * S + 2048]
        if o_idx not in SCHRAUD:
            # Z = Zh0 + Zh1 from the two ACT accumulator reads
            nc.vector.tensor_tensor(
                stats[:, sb + 2 : sb + 3], stats[:, sb : sb + 1],
                stats[:, sb + 1 : sb + 2], mybir.AluOpType.add,
            )
        else:
            nc.vector.scalar_tensor_tensor(
                zscrV[:], e0, 1.0, e1,
                mybir.AluOpType.mult, mybir.AluOpType.add,
                accum_out=stats[:, sb + 2 : sb + 3],
            )
        nc.vector.reciprocal(stats[:, sb + 3 : sb + 4], stats[:, sb + 2 : sb + 3])
        vb = cid * D
        nc.vector.tensor_scalar_mul(
            Vs[:, vb : vb + D], V[:, vb : vb + D], stats[:, sb + 3 : sb + 4]
        )

    drainers = []

    def emit_av(i, last=False):
        """O^T[(b,d), q] += Vs_i^T @ E_i; b0/b1 in disjoint PE column strips."""
        for j in range(4):
            for b in range(BPC):
                cid = b * NCH + i
                vb = cid * D
                eb = cid * S
                mm = nc.tensor.matmul(
                    pot[b * 64 : (b + 1) * 64, j * 512 : (j + 1) * 512],
                    lhsT=Vs[:, vb : vb + D],
                    rhs=E[:, eb + j * 512 : eb + (j + 1) * 512],
                    start=False,
                    stop=(last and b == BPC - 1),
                    skip_group_check=True,
                )
                if not zmm_linked[j]:
                    add_dep_helper(
                        mm.ins, zmm[j].ins, sync=False,
                        reason="AV accumulation after bank-opening zero matmul",
                    )
            if last:
                drainers.append(j)
                emit_drain(j)

    def emit_drain(j):
        """Unpack pot q-block j, transpose to natural layout, store."""
        if j % 2 == 0:
            nc.vector.tensor_copy(
                OT[:, j * 512 : (j + 1) * 512], pot[:, j * 512 : (j + 1) * 512]
            )
        else:
            nc.scalar.copy(
                OT[:, j * 512 : (j + 1) * 512], pot[:, j * 512 : (j + 1) * 512]
            )
        o_view = O_all[:].rearrange("p (m b d) -> p m b d", m=NCH, b=BPC, d=D)
        for m in range(4 * j, 4 * j + 4):
            ptc = ps.tile([128, 128], BF16, tag="ps", name=f"ptc_{m}")
            nc.tensor.transpose(ptc[:], OT[:, m * 128 : (m + 1) * 128], identb[:])
            if m % 2 == 0:
                nc.vector.tensor_copy(O_all[:, m * 128 : (m + 1) * 128], ptc[:])
            else:
                nc.scalar.copy(O_all[:, m * 128 : (m + 1) * 128], ptc[:])
        for b in range(BPC):
            eng = nc.sync if b == 0 else nc.gpsimd
            eng.dma_start(
                o[b, 4 * j * 128 : (4 * j + 4) * 128, :].rearrange(
                    "(m p) d -> p m d", p=128
                ),
                o_view[:, 4 * j : 4 * j + 4, b, :],
            )

    zmm_linked = [False, False, False, False]

    # -- schedule --
    # i=0..2: H0 only (QT half 1 arrives via DMA roundtrip a bit later)
    for i in range(3):
        for b in range(BPC):
            emit_unit(i, b, 0)
    # i=3: first full chunk; open the accumulator banks in-stream
    for b in range(BPC):
        emit_unit(3, b, 0)
        emit_unit(3, b, 1)
        emit_zfold(3, b)
    emit_zmm()
    for i in (4, 5):
        for b in range(BPC):
            emit_unit(i, b, 0)
            emit_unit(i, b, 1)
            emit_zfold(i, b)
    emit_av(3)
    zmm_linked = [True] * 4

    # steady state: scores/exp for chunk i, AV for chunk i-2 (two-chunk lag
    # so the exp -> Z -> 1/Z -> V-fold chain never stalls the PE); weave the
    # deferred H1 halves of chunks 0..2 (and their AVs) into i=6..11.
    for i in range(6, NCH):
        for b in range(BPC):
            emit_unit(i, b, 0)
            emit_unit(i, b, 1)
            emit_zfold(i, b)
        emit_av(i - 2)
        if i in (6, 8, 10):
            si = (i - 6) // 2
            for b in range(BPC):
                emit_unit(si, b, 1)
                emit_zfold(si, b)
        elif i in (7, 9, 11):
            emit_av((i - 7) // 2)
    emit_av(NCH - 2)
    emit_av(NCH - 1, last=True)


_CACHE: dict = {}


def build_program():
    if "nc" in _CACHE:
        return _CACHE["nc"]
    nc = bacc.Bacc("TRN2", target_bir_lowering=False, debug=False)
    q = nc.dram_tensor("q", [BPC, S, D], FP32, kind="ExternalInput").ap()
    k = nc.dram_tensor("k", [BPC, S, D], FP32, kind="ExternalInput").ap()
    v = nc.dram_tensor("v", [BPC, S, D], FP32, kind="ExternalInput").ap()
    o = nc.dram_tensor("o", [BPC, S, D], FP32, kind="ExternalOutput").ap()
    with tile.TileContext(nc) as tc:
        with ExitStack() as ctx:
            emit_kernel(ctx, tc, q, k, v, o)
    nc.compile()
    _CACHE["nc"] = nc
    return nc


def make_in_maps(q, k, v):
    q = np.ascontiguousarray(q, dtype=np.float32)
    k = np.ascontiguousarray(k, dtype=np.float32)
    v = np.ascontiguousarray(v, dtype=np.float32)
    assert q.shape == (B_FULL, S, D), q.shape
    return [
        {
            "q": np.ascontiguousarray(q[c * BPC : (c + 1) * BPC]),
            "k": np.ascontiguousarray(k[c * BPC : (c + 1) * BPC]),
            "v": np.ascontiguousarray(v[c * BPC : (c + 1) * BPC]),
        }
        for c in range(N_CORES)
    ]


def kernel(q, k, v, _trace=False):
    nc = build_program()
    in_maps = make_in_maps(q, k, v)
    try:
        res = bass_utils.run_bass_kernel_spmd(
            nc, in_maps, core_ids=list(range(N_CORES)), trace=_trace
        )
    except Exception:
        # This box occasionally throws a transient device error
        # (NRT_EXEC_UNIT_UNRECOVERABLE); a short pause + one retry has
        # recovered it every time it was observed.
        import time

        time.sleep(5)
        res = bass_utils.run_bass_kernel_spmd(
            nc, in_maps, core_ids=list(range(N_CORES)), trace=_trace
        )
    out = np.concatenate([r["o"] for r in res.results], axis=0)
    if _trace:
        return out, res
    return out


# revision 25
# speedup vs baseline: 1.1422x; 1.1111x over previous
"""TRN2 Bass/Tile kernel for nn_DotProductAttention (softmax over the QUERY axis).

reference:
    scores  = einsum('bqd,bkd->bqk', q, k) / sqrt(64)
    weights = softmax(scores, axis=1)          # over q, NOT k!
    out     = einsum('bqk,bkd->bqd', weights, v)

Because the softmax normalizes over q for each (b, k) column, we work with the
transposed score matrix T = K @ Q^T (shape [k, q]): the reduction axis (q) is
then the free axis, which the ACT accum_out reduction handles for free, and the
normalizer Z[k] lives on the contraction axis of the second matmul so it can be
folded into V (V' = V / Z) instead of rescaling the whole [k, q] tile.

Sharding: B=16 batches, data-parallel over 8 cores => 2 batches per core.
The two batches of a core are packed into the two 64-partition halves of
[128, *] tiles ((b, d) packing), which lets pairs of matmuls run concurrently
in disjoint PE-array row strips (scores) / column strips (AV).
"""

import math
from contextlib import ExitStack

import numpy as np

import concourse.bass as bass  # noqa: F401  (kept for symmetry with docs)
import concourse.mybir as mybir
import concourse.tile as tile
from bass_rust import add_dep_helper
from concourse import bacc, bass_utils
from concourse.masks import make_identity

FP32 = mybir.dt.float32
BF16 = mybir.dt.bfloat16

N_CORES = 8
B_FULL = 16
BPC = B_FULL // N_CORES  # batches per core = 2
S = 2048
D = 64
NCH = S // 128  # 16 key chunks of 128
SCALE = 1.0 / math.sqrt(D)


def emit_kernel(ctx: ExitStack, tc, q, k, v, o):
    """Emit the per-core Tile program. q/k/v/o are DRAM APs of [BPC, S, D] f32."""
    nc = tc.nc

    const_pool = ctx.enter_context(tc.tile_pool(name="const", bufs=1))
    big = ctx.enter_context(tc.tile_pool(name="big", bufs=1))
    dram = ctx.enter_context(tc.tile_pool(name="dram", bufs=1, space="DRAM"))
    # PSUM: phase B1 double-buffers two [128,2048] score tiles (all 8 banks);
    # phase B2 reuses the same pool for the [128,2048] O^T accumulator and
    # the [128,128] transpose tiles.
    ps = ctx.enter_context(tc.tile_pool(name="ps", bufs=2, space="PSUM"))

    identb = const_pool.tile([128, 128], BF16, name="identb")
    make_identity(nc, identb)
    zw = const_pool.tile([128, 128], BF16, name="zw")
    nc.vector.memset(zw[:], 0.0)

    # (b,d)-packed transposed operands: partitions 0:64 = batch0 d, 64:128 = batch1 d.
    QT = big.tile([128, S], BF16, name="QT")
    KT = big.tile([128, S], BF16, name="KT")
    # staging for Q/K chunks in (m, b, d) column layout, s on partitions
    qstage = big.tile([128, S], FP32, name="qstage")
    kstage = big.tile([128, S], FP32, name="kstage")
    # V chunks [128 k, 64 d] (f32 as loaded) and Vs = V / Z (bf16)
    V = big.tile([128, BPC * NCH * D], FP32, name="V")
    Vs = big.tile([128, BPC * NCH * D], BF16, name="Vs")
    # per (b, chunk) stats columns: [z, 1/z]
    stats = big.tile([128, BPC * NCH * 2], FP32, name="stats")
    # E[(b*NCH+i)*S :+ S] = exp(scores/sqrt(D)): [128 k, 2048 q] bf16, fully resident
    E = big.tile([128, BPC * NCH * S], BF16, name="E")
    # O^T staging ((b,d) packed on partitions, q on free), bf16 so the
    # output transposes run on the cheap bf16 path (~0.4% output rounding,
    # well inside the 2e-2 gate)
    OT = big.tile([128, S], BF16, name="OT")
    # O in natural layout: column chunk m holds [q-tile m, (b d)]
    O_all = big.tile([128, S], FP32, name="O_all")
    # dummy destination for the DVE-side normalizer reductions
    zscr = big.tile([128, S], BF16, name="zscr")

    # bf16 copies of the staged (s, (b d)) matrices
    qbf = big.tile([128, S], BF16, name="qbf")
    kbf = big.tile([128, S], BF16, name="kbf")
    kbf_dram = dram.tile([S, 128], BF16, name="kbf_dram")
    qbf_dram = dram.tile([S // 2, 128], BF16, name="qbf_dram")

    # ---------------- phase A: load + transpose Q/K, load V ----------------
    # Loads: quarter-granularity strided stage DMAs, batch 0 on the sync
    # HWDGE queue and batch 1 on the scalar queue; V on the otherwise-idle
    # SWDGE (gpsimd) path. Transposes: chunk 0's scores read ALL of QT plus
    # KT chunk 0, so q0..15+k0 go through fast PE transposes (bf16, psum)
    # with the copies on the pre-B1-idle ACT engine; KT chunks 1..15 arrive
    # slightly later via one whole-tensor xbar DMA (bf16 DRAM roundtrip).
    # All stage DMAs ride the sync queue: the scalar queue must stay clear for
    # the transpose copies + ACTIVATEs (a stalled DMA there blocks B1), and q
    # loads fully before k so chunk 0 can start earliest.
    QRT = NCH // 4  # chunks per quarter-DMA
    last_q_dma = None
    # k's first quarter is needed by the transpose chain before q's last two
    # quarters, so interleave it into the load stream
    for src, stg, Q in (
        (q, qstage, 0),
        (q, qstage, 1),
        (k, kstage, 0),
        (q, qstage, 2),
        (q, qstage, 3),
        (k, kstage, 1),
        (k, kstage, 2),
        (k, kstage, 3),
    ):
        ssl = slice(Q * QRT * 128, (Q + 1) * QRT * 128)
        for b in range(BPC):
            dma = nc.sync.dma_start(
                stg[:, ssl].rearrange("p (m b d) -> p m b d", m=QRT, b=BPC, d=D)[
                    :, :, b, :
                ],
                src[b, ssl, :].rearrange("(m p) d -> p m d", p=128),
            )
            if src is q:
                last_q_dma = dma
    for b in range(BPC):
        vdma = nc.gpsimd.dma_start(
            V[:].rearrange("p (b m d) -> p b m d", b=BPC, m=NCH)[:, b, :, :],
            v[b].rearrange("(m p) d -> p m d", p=128),
        )
        # V isn't needed until well into B1 — keep its HBM traffic out of the
        # way of the critical q loads
        add_dep_helper(
            vdma.ins, last_q_dma.ins, sync=True, reason="delay V behind q loads"
        )
    for Q in range(4):
        csl = slice(Q * QRT * 128, (Q + 1) * QRT * 128)
        nc.vector.tensor_copy(qbf[:, csl], qstage[:, csl])
        nc.vector.tensor_copy(kbf[:, csl], kstage[:, csl])
    # PE-transposed chunks: only the ones B1 needs before the DRAM-roundtrip
    # xbar transposes can deliver the rest (q8..15 and k4..15 ride the xbar).
    for idx, (t, m) in enumerate(
        [("q", mm) for mm in range(8)] + [("k", mm) for mm in range(4)]
    ):
        bft, dst = (qbf, QT) if t == "q" else (kbf, KT)
        pt = ps.tile([128, 128], BF16, tag="ps", name=f"pt_{t}{m}")
        nc.tensor.transpose(pt[:], bft[:, m * 128 : (m + 1) * 128], identb[:])
        # alternate the psum-drain copies between the two idle engines so the
        # copy stage isn't the chain's rate limiter
        if idx % 2 == 0:
            nc.scalar.copy(dst[:, m * 128 : (m + 1) * 128], pt[:])
        else:
            nc.vector.tensor_copy(dst[:, m * 128 : (m + 1) * 128], pt[:])
    # KT chunks 4..15 / QT chunks 8..15 via DRAM-roundtrip xbar transposes
    nc.sync.dma_start(
        kbf_dram[512:S, :].rearrange("(m p) c -> p m c", p=128),
        kbf[:, 512:S].rearrange("p (m c) -> p m c", m=NCH - 4),
    )
    nc.sync.dma_start(
        qbf_dram[:].rearrange("(m p) c -> p m c", p=128),
        qbf[:, 1024:S].rearrange("p (m c) -> p m c", m=NCH - 8),
    )
    nc.sync.dma_start_transpose(out=KT[:, 512:S], in_=kbf_dram[512:S, :])
    nc.sync.dma_start_transpose(out=QT[:, 1024:S], in_=qbf_dram[:])

    # ---------------- phase B1: scores -> exp, double-buffered --------------
    # Two [128,2048] score tiles rotate through all 8 PSUM banks; one N=2048
    # exp per (batch, chunk) with accum_out giving the softmax normalizer Z
    # directly (softmax axis == free axis).
    for i in range(NCH):
        for b in range(BPC):
            sct = ps.tile([128, S], FP32, tag="ps", name=f"sc{i}_{b}")
            for j in range(4):
                nc.tensor.matmul(
                    sct[:, j * 512 : (j + 1) * 512],
                    lhsT=KT[b * 64 : (b + 1) * 64, i * 128 : (i + 1) * 128],
                    rhs=QT[b * 64 : (b + 1) * 64, j * 512 : (j + 1) * 512],
                    start=True,
                    stop=True,
                )
            sb = (b * NCH + i) * 2
            eb = (b * NCH + i) * S
            if i < 11:
                # Z for early chunks isn't needed until B2, and the DVE is
                # mostly idle during B1 — reduce E there and spare the ACT
                # queue the 286ns accumulator read.  Fold tree (two 2x-rate
                # bf16 adds + a short reduce, 1.8us) instead of a straight
                # 1x FD=2048 reduce (2.28us).
                nc.scalar.activation(
                    E[:, eb : eb + S],
                    sct[:],
                    mybir.ActivationFunctionType.Exp,
                    scale=SCALE,
                )
                nc.vector.tensor_add(
                    zscr[:, 0:1024], E[:, eb : eb + 1024], E[:, eb + 1024 : eb + S]
                )
                nc.vector.tensor_add(
                    zscr[:, 1024:1536], zscr[:, 0:512], zscr[:, 512:1024]
                )
                nc.vector.tensor_reduce(
                    stats[:, sb : sb + 1],
                    zscr[:, 1024:1536],
                    mybir.AxisListType.X,
                    mybir.AluOpType.add,
                )
            else:
                nc.scalar.activation(
                    E[:, eb : eb + S],
                    sct[:],
                    mybir.ActivationFunctionType.Exp,
                    scale=SCALE,
                    accum_out=stats[:, sb : sb + 1],
                )
            vb = (b * NCH + i) * D
            nc.vector.reciprocal(stats[:, sb + 1 : sb + 2], stats[:, sb : sb + 1])
            nc.vector.tensor_scalar_mul(
                Vs[:, vb : vb + D], V[:, vb : vb + D], stats[:, sb + 1 : sb + 2]
            )
    # ---------------- phase B2: dense AV accumulation -----------------------
    # Open the accumulator: each bank gets a full-128-partition zeroing matmul
    # (zero weights) that writes 0 everywhere and sets has_written for the
    # whole bank on every execution, so the partition-sliced AV matmuls can
    # all accumulate with start=False regardless of how the HW scopes the
    # first_mm bank-clear across partitions.
    pot = ps.tile([128, S], FP32, tag="ps", name="pot")
    zmm = []
    for j in range(4):
        zmm.append(
            nc.tensor.matmul(
                pot[:, j * 512 : (j + 1) * 512],
                lhsT=zw[:],
                rhs=QT[:, 0:512],
                start=True,
                stop=False,
                skip_group_check=True,
            )
        )
    o_view = O_all[:].rearrange("p (m b d) -> p m b d", m=NCH, b=BPC, d=D)

    def emit_av(i, j):
        for b in range(BPC):
            # O^T[(b,d), q] += Vs_i^T @ E_i ; b0 -> PE cols 0:63,
            # b1 -> cols 64:127 (concurrent via col tiling)
            vb = (b * NCH + i) * D
            eb = (b * NCH + i) * S
            mm = nc.tensor.matmul(
                pot[b * 64 : (b + 1) * 64, j * 512 : (j + 1) * 512],
                lhsT=Vs[:, vb : vb + D],
                rhs=E[:, eb + j * 512 : eb + (j + 1) * 512],
                start=False,
                stop=(i == NCH - 1 and b == BPC - 1),
                skip_group_check=True,
            )
            if i == 0:
                add_dep_helper(
                    mm.ins,
                    zmm[j].ins,
                    sync=False,
                    reason="AV accumulation after bank-opening zero matmul",
                )

    # dense i-outer accumulation; each region is unpacked (ACT engine) right
    # after its closing matmul so pot's psum slot frees as early as possible —
    # then the 16 out-transposes run with BOTH slots available (2-deep)
    # instead of serializing against the pot-pinned pool.
    for i in range(NCH - 1):
        for j in range(4):
            emit_av(i, j)
    for j in range(4):
        emit_av(NCH - 1, j)
        # alternate the unpacks across ACT and DVE so they drain in parallel
        if j % 2 == 0:
            nc.vector.tensor_copy(
                OT[:, j * 512 : (j + 1) * 512], pot[:, j * 512 : (j + 1) * 512]
            )
        else:
            nc.scalar.copy(
                OT[:, j * 512 : (j + 1) * 512], pot[:, j * 512 : (j + 1) * 512]
            )
    for j in range(4):
        for m in range(4 * j, 4 * j + 4):
            ptc = ps.tile([128, 128], BF16, tag="ps", name=f"ptc_{m}")
            nc.tensor.transpose(ptc[:], OT[:, m * 128 : (m + 1) * 128], identb[:])
            if m % 2 == 0:
                nc.vector.tensor_copy(O_all[:, m * 128 : (m + 1) * 128], ptc[:])
            else:
                nc.scalar.copy(O_all[:, m * 128 : (m + 1) * 128], ptc[:])
        for b in range(BPC):
            # four trigger queues so the four j-blocks' stores all drain
            # concurrently at the tail (ACT/DVE are done by the time their
            # triggers fire)
            if b == 0:
                eng = nc.sync if j % 2 == 0 else nc.scalar
            else:
                eng = nc.gpsimd
            eng.dma_start(
                o[b, 4 * j * 128 : (4 * j + 4) * 128, :].rearrange(
                    "(m p) d -> p m d", p=128
                ),
                o_view[:, 4 * j : 4 * j + 4, b, :],
            )


_CACHE: dict = {}


def build_program():
    if "nc" in _CACHE:
        return _CACHE["nc"]
    nc = bacc.Bacc("TRN2", target_bir_lowering=False, debug=False)
    q = nc.dram_tensor("q", [BPC, S, D], FP32, kind="ExternalInput").ap()
    k = nc.dram_tensor("k", [BPC, S, D], FP32, kind="ExternalInput").ap()
    v = nc.dram_tensor("v", [BPC, S, D], FP32, kind="ExternalInput").ap()
    o = nc.dram_tensor("o", [BPC, S, D], FP32, kind="ExternalOutput").ap()
    with tile.TileContext(nc) as tc:
        with ExitStack() as ctx:
            emit_kernel(ctx, tc, q, k, v, o)
    nc.compile()
    _CACHE["nc"] = nc
    return nc


def make_in_maps(q, k, v):
    q = np.ascontiguousarray(q, dtype=np.float32)
    k = np.ascontiguousarray(k, dtype=np.float32)
    v = np.ascontiguousarray(v, dtype=np.float32)
    assert q.shape == (B_FULL, S, D), q.shape
    return [
        {
            "q": np.ascontiguousarray(q[c * BPC : (c + 1) * BPC]),
            "k": np.ascontiguousarray(k[c * BPC : (c + 1) * BPC]),
            "v": np.ascontiguousarray(v[c * BPC : (c + 1) * BPC]),
        }
        for c in range(N_CORES)
    ]


def kernel(q, k, v, _trace=False):
    nc = build_program()
    in_maps = make_in_maps(q, k, v)
    try:
        res = bass_utils.run_bass_kernel_spmd(
            nc, in_maps, core_ids=list(range(N_CORES)), trace=_trace
        )
    except Exception:
        # This box occasionally throws a transient device error
        # (NRT_EXEC_UNIT_UNRECOVERABLE); a short pause + one retry has
        # recovered it every time it was observed.
        import time

        time.sleep(5)
        res = bass_utils.run_bass_kernel_spmd(
            nc, in_maps, core_ids=list(range(N_CORES)), trace=_trace
        )
    out = np.concatenate([r["o"] for r in res.results], axis=0)
    if _trace:
        return out, res
    return out



# revision 26
# speedup vs baseline: 1.2484x; 1.0929x over previous
"""TRN2 Bass/Tile kernel for nn_DotProductAttention (softmax over the QUERY axis).

reference:
    scores  = einsum('bqd,bkd->bqk', q, k) / sqrt(64)
    weights = softmax(scores, axis=1)          # over q, NOT k!
    out     = einsum('bqk,bkd->bqd', weights, v)

Because the softmax normalizes over q for each (b, k) column, we work with the
transposed score matrix T = K @ Q^T (shape [k, q]): the reduction axis (q) is
then the free axis, which the ACT accum_out reduction handles for free, and the
normalizer Z[k] lives on the contraction axis of the second matmul so it can be
folded into V (V' = V / Z) instead of rescaling the whole [k, q] tile.

Sharding: B=16 batches, data-parallel over 8 cores => 2 batches per core.
The two batches of a core are packed into the two 64-partition halves of
[128, *] tiles ((b, d) packing), which lets pairs of matmuls run concurrently
in disjoint PE-array row strips (scores) / column strips (AV).
"""

import math
from contextlib import ExitStack

import numpy as np

import concourse.bass as bass  # noqa: F401  (kept for symmetry with docs)
import concourse.mybir as mybir
import concourse.tile as tile
from bass_rust import add_dep_helper
from concourse import bacc, bass_utils
from concourse.masks import make_identity

FP32 = mybir.dt.float32
BF16 = mybir.dt.bfloat16

N_CORES = 8
B_FULL = 16
BPC = B_FULL // N_CORES  # batches per core = 2
S = 2048
D = 64
NCH = S // 128  # 16 key chunks of 128
SCALE = 1.0 / math.sqrt(D)


def emit_kernel(ctx: ExitStack, tc, q, k, v, o):
    """Emit the per-core Tile program. q/k/v/o are DRAM APs of [BPC, S, D] f32."""
    nc = tc.nc

    const_pool = ctx.enter_context(tc.tile_pool(name="const", bufs=1))
    big = ctx.enter_context(tc.tile_pool(name="big", bufs=1))
    dram = ctx.enter_context(tc.tile_pool(name="dram", bufs=1, space="DRAM"))
    # PSUM: phase B1 double-buffers two [128,2048] score tiles (all 8 banks);
    # phase B2 reuses the same pool for the [128,2048] O^T accumulator and
    # the [128,128] transpose tiles.
    ps = ctx.enter_context(tc.tile_pool(name="ps", bufs=2, space="PSUM"))

    identb = const_pool.tile([128, 128], BF16, name="identb")
    make_identity(nc, identb)
    zw = const_pool.tile([128, 128], BF16, name="zw")
    nc.vector.memset(zw[:], 0.0)

    # (b,d)-packed transposed operands: partitions 0:64 = batch0 d, 64:128 = batch1 d.
    QT = big.tile([128, S], BF16, name="QT")
    KT = big.tile([128, S], BF16, name="KT")
    # staging for Q/K chunks in (m, b, d) column layout, s on partitions
    qstage = big.tile([128, S], FP32, name="qstage")
    kstage = big.tile([128, S], FP32, name="kstage")
    # V chunks [128 k, 64 d] (f32 as loaded) and Vs = V / Z (bf16)
    V = big.tile([128, BPC * NCH * D], FP32, name="V")
    Vs = big.tile([128, BPC * NCH * D], BF16, name="Vs")
    # per (b, chunk) stats columns: [z, 1/z]
    stats = big.tile([128, BPC * NCH * 2], FP32, name="stats")
    # E[(b*NCH+i)*S :+ S] = exp(scores/sqrt(D)): [128 k, 2048 q] bf16, fully resident
    E = big.tile([128, BPC * NCH * S], BF16, name="E")
    # O^T staging ((b,d) packed on partitions, q on free), bf16 so the
    # output transposes run on the cheap bf16 path (~0.4% output rounding,
    # well inside the 2e-2 gate)
    OT = big.tile([128, S], BF16, name="OT")
    # O in natural layout: column chunk m holds [q-tile m, (b d)]
    O_all = big.tile([128, S], FP32, name="O_all")
    # dummy destination for the DVE-side normalizer reductions
    zscr = big.tile([128, S], BF16, name="zscr")

    # bf16 copies of the staged (s, (b d)) matrices
    qbf = big.tile([128, S], BF16, name="qbf")
    kbf = big.tile([128, S], BF16, name="kbf")
    kbf_dram = dram.tile([S, 128], BF16, name="kbf_dram")

    # ---------------- phase A: load + transpose Q/K, load V ----------------
    # Loads: quarter-granularity strided stage DMAs, batch 0 on the sync
    # HWDGE queue and batch 1 on the scalar queue; V on the otherwise-idle
    # SWDGE (gpsimd) path. Transposes: chunk 0's scores read ALL of QT plus
    # KT chunk 0, so q0..15+k0 go through fast PE transposes (bf16, psum)
    # with the copies on the pre-B1-idle ACT engine; KT chunks 1..15 arrive
    # slightly later via one whole-tensor xbar DMA (bf16 DRAM roundtrip).
    # All stage DMAs ride the sync queue: the scalar queue must stay clear for
    # the transpose copies + ACTIVATEs (a stalled DMA there blocks B1), and q
    # loads fully before k so chunk 0 can start earliest.
    QRT = NCH // 4  # chunks per quarter-DMA
    last_q_dma = None
    # k's first quarter is needed by the transpose chain before q's last two
    # quarters, so interleave it into the load stream
    for src, stg, Q in (
        (q, qstage, 0),
        (q, qstage, 1),
        (k, kstage, 0),
        (q, qstage, 2),
        (q, qstage, 3),
        (k, kstage, 1),
        (k, kstage, 2),
        (k, kstage, 3),
    ):
        ssl = slice(Q * QRT * 128, (Q + 1) * QRT * 128)
        for b in range(BPC):
            dma = nc.sync.dma_start(
                stg[:, ssl].rearrange("p (m b d) -> p m b d", m=QRT, b=BPC, d=D)[
                    :, :, b, :
                ],
                src[b, ssl, :].rearrange("(m p) d -> p m d", p=128),
            )
            if src is q:
                last_q_dma = dma
    for b in range(BPC):
        vdma = nc.gpsimd.dma_start(
            V[:].rearrange("p (b m d) -> p b m d", b=BPC, m=NCH)[:, b, :, :],
            v[b].rearrange("(m p) d -> p m d", p=128),
        )
        # V isn't needed until well into B1 — keep its HBM traffic out of the
        # way of the critical q loads
        add_dep_helper(
            vdma.ins, last_q_dma.ins, sync=True, reason="delay V behind q loads"
        )
    for Q in range(4):
        csl = slice(Q * QRT * 128, (Q + 1) * QRT * 128)
        nc.vector.tensor_copy(qbf[:, csl], qstage[:, csl])
        nc.vector.tensor_copy(kbf[:, csl], kstage[:, csl])
    # PE-transposed chunks: q0..15 then k0..3 (the chunks B1 needs before the
    # DRAM-roundtrip xbar below can deliver the rest of KT)
    for idx, (t, m) in enumerate(
        [("q", mm) for mm in range(8)]
        + [("k", mm) for mm in range(4)]
        + [("q", mm) for mm in range(8, NCH)]
    ):
        bft, dst = (qbf, QT) if t == "q" else (kbf, KT)
        pt = ps.tile([128, 128], BF16, tag="ps", name=f"pt_{t}{m}")
        nc.tensor.transpose(pt[:], bft[:, m * 128 : (m + 1) * 128], identb[:])
        # alternate the psum-drain copies between the two idle engines so the
        # copy stage isn't the chain's rate limiter
        if idx % 2 == 0:
            nc.scalar.copy(dst[:, m * 128 : (m + 1) * 128], pt[:])
        else:
            nc.vector.tensor_copy(dst[:, m * 128 : (m + 1) * 128], pt[:])
    # KT chunks 4..15 via DRAM-roundtrip whole-tensor xbar transpose
    nc.sync.dma_start(
        kbf_dram[512:S, :].rearrange("(m p) c -> p m c", p=128),
        kbf[:, 512:S].rearrange("p (m c) -> p m c", m=NCH - 4),
    )
    nc.sync.dma_start_transpose(out=KT[:, 512:S], in_=kbf_dram[512:S, :])

    # ---------------- phase B1: scores -> exp, double-buffered --------------
    # Two [128,2048] score tiles rotate through all 8 PSUM banks; one N=2048
    # exp per (batch, chunk) with accum_out giving the softmax normalizer Z
    # directly (softmax axis == free axis).
    for i in range(NCH):
        for b in range(BPC):
            sct = ps.tile([128, S], FP32, tag="ps", name=f"sc{i}_{b}")
            for j in range(4):
                nc.tensor.matmul(
                    sct[:, j * 512 : (j + 1) * 512],
                    lhsT=KT[b * 64 : (b + 1) * 64, i * 128 : (i + 1) * 128],
                    rhs=QT[b * 64 : (b + 1) * 64, j * 512 : (j + 1) * 512],
                    start=True,
                    stop=True,
                )
            sb = (b * NCH + i) * 2
            eb = (b * NCH + i) * S
            if i < 11:
                # Z for early chunks isn't needed until B2, and the DVE is
                # mostly idle during B1 — reduce E there and spare the ACT
                # queue the 286ns accumulator read.  Fold tree (two 2x-rate
                # bf16 adds + a short reduce, 1.8us) instead of a straight
                # 1x FD=2048 reduce (2.28us).
                nc.scalar.activation(
                    E[:, eb : eb + S],
                    sct[:],
                    mybir.ActivationFunctionType.Exp,
                    scale=SCALE,
                )
                nc.vector.tensor_add(
                    zscr[:, 0:1024], E[:, eb : eb + 1024], E[:, eb + 1024 : eb + S]
                )
                nc.vector.tensor_add(
                    zscr[:, 1024:1536], zscr[:, 0:512], zscr[:, 512:1024]
                )
                nc.vector.tensor_reduce(
                    stats[:, sb : sb + 1],
                    zscr[:, 1024:1536],
                    mybir.AxisListType.X,
                    mybir.AluOpType.add,
                )
            else:
                nc.scalar.activation(
                    E[:, eb : eb + S],
                    sct[:],
                    mybir.ActivationFunctionType.Exp,
                    scale=SCALE,
                    accum_out=stats[:, sb : sb + 1],
                )
            vb = (b * NCH + i) * D
            nc.vector.reciprocal(stats[:, sb + 1 : sb + 2], stats[:, sb : sb + 1])
            nc.vector.tensor_scalar_mul(
                Vs[:, vb : vb + D], V[:, vb : vb + D], stats[:, sb + 1 : sb + 2]
            )
    # ---------------- phase B2: dense AV accumulation -----------------------
    # Open the accumulator: each bank gets a full-128-partition zeroing matmul
    # (zero weights) that writes 0 everywhere and sets has_written for the
    # whole bank on every execution, so the partition-sliced AV matmuls can
    # all accumulate with start=False regardless of how the HW scopes the
    # first_mm bank-clear across partitions.
    pot = ps.tile([128, S], FP32, tag="ps", name="pot")
    zmm = []
    for j in range(4):
        zmm.append(
            nc.tensor.matmul(
                pot[:, j * 512 : (j + 1) * 512],
                lhsT=zw[:],
                rhs=QT[:, 0:512],
                start=True,
                stop=False,
                skip_group_check=True,
            )
        )
    o_view = O_all[:].rearrange("p (m b d) -> p m b d", m=NCH, b=BPC, d=D)

    def emit_av(i, j):
        for b in range(BPC):
            # O^T[(b,d), q] += Vs_i^T @ E_i ; b0 -> PE cols 0:63,
            # b1 -> cols 64:127 (concurrent via col tiling)
            vb = (b * NCH + i) * D
            eb = (b * NCH + i) * S
            mm = nc.tensor.matmul(
                pot[b * 64 : (b + 1) * 64, j * 512 : (j + 1) * 512],
                lhsT=Vs[:, vb : vb + D],
                rhs=E[:, eb + j * 512 : eb + (j + 1) * 512],
                start=False,
                stop=(i == NCH - 1 and b == BPC - 1),
                skip_group_check=True,
            )
            if i == 0:
                add_dep_helper(
                    mm.ins,
                    zmm[j].ins,
                    sync=False,
                    reason="AV accumulation after bank-opening zero matmul",
                )

    # dense i-outer accumulation; each region is unpacked (ACT engine) right
    # after its closing matmul so pot's psum slot frees as early as possible —
    # then the 16 out-transposes run with BOTH slots available (2-deep)
    # instead of serializing against the pot-pinned pool.
    for i in range(NCH - 1):
        for j in range(4):
            emit_av(i, j)
    for j in range(4):
        emit_av(NCH - 1, j)
        # alternate the unpacks across ACT and DVE so they drain in parallel
        if j % 2 == 0:
            nc.vector.tensor_copy(
                OT[:, j * 512 : (j + 1) * 512], pot[:, j * 512 : (j + 1) * 512]
            )
        else:
            nc.scalar.copy(
                OT[:, j * 512 : (j + 1) * 512], pot[:, j * 512 : (j + 1) * 512]
            )
    for j in range(4):
        for m in range(4 * j, 4 * j + 4):
            ptc = ps.tile([128, 128], BF16, tag="ps", name=f"ptc_{m}")
            nc.tensor.transpose(ptc[:], OT[:, m * 128 : (m + 1) * 128], identb[:])
            if m % 2 == 0:
                nc.vector.tensor_copy(O_all[:, m * 128 : (m + 1) * 128], ptc[:])
            else:
                nc.scalar.copy(O_all[:, m * 128 : (m + 1) * 128], ptc[:])
        for b in range(BPC):
            # batch 1 outputs ride the idle SWDGE ring so the two batches'
            # stores drain concurrently at the tail; batch 0 alternates between
            # the sync and scalar trigger queues so the four j-block stores
            # overlap pairwise as well
            eng = (nc.sync if j % 2 == 0 else nc.scalar) if b == 0 else nc.gpsimd
            eng.dma_start(
                o[b, 4 * j * 128 : (4 * j + 4) * 128, :].rearrange(
                    "(m p) d -> p m d", p=128
                ),
                o_view[:, 4 * j : 4 * j + 4, b, :],
            )


_CACHE: dict = {}


def build_program():
    if "nc" in _CACHE:
        return _CACHE["nc"]
    nc = bacc.Bacc("TRN2", target_bir_lowering=False, debug=False)
    q = nc.dram_tensor("q", [BPC, S, D], FP32, kind="ExternalInput").ap()
    k = nc.dram_tensor("k", [BPC, S, D], FP32, kind="ExternalInput").ap()
    v = nc.dram_tensor("v", [BPC, S, D], FP32, kind="ExternalInput").ap()
    o = nc.dram_tensor("o", [BPC, S, D], FP32, kind="ExternalOutput").ap()
    with tile.TileContext(nc) as tc:
        with ExitStack() as ctx:
            emit_kernel(ctx, tc, q, k, v, o)
    nc.compile()
    _CACHE["nc"] = nc
    return nc


def make_in_maps(q, k, v):
    q = np.ascontiguousarray(q, dtype=np.float32)
    k = np.ascontiguousarray(k, dtype=np.float32)
    v = np.ascontiguousarray(v, dtype=np.float32)
    assert q.shape == (B_FULL, S, D), q.shape
    return [
        {
            "q": np.ascontiguousarray(q[c * BPC : (c + 1) * BPC]),
            "k": np.ascontiguousarray(k[c * BPC : (c + 1) * BPC]),
            "v": np.ascontiguousarray(v[c * BPC : (c + 1) * BPC]),
        }
        for c in range(N_CORES)
    ]


def kernel(q, k, v, _trace=False):
    nc = build_program()
    in_maps = make_in_maps(q, k, v)
    try:
        res = bass_utils.run_bass_kernel_spmd(
            nc, in_maps, core_ids=list(range(N_CORES)), trace=_trace
        )
    except Exception:
        # This box occasionally throws a transient device error
        # (NRT_EXEC_UNIT_UNRECOVERABLE); a short pause + one retry has
        # recovered it every time it was observed.
        import time

        time.sleep(5)
        res = bass_utils.run_bass_kernel_spmd(
            nc, in_maps, core_ids=list(range(N_CORES)), trace=_trace
        )
    out = np.concatenate([r["o"] for r in res.results], axis=0)
    if _trace:
        return out, res
    return out



# revision 32
# speedup vs baseline: 1.2512x; 1.0022x over previous
"""TRN2 Bass/Tile kernel for nn_DotProductAttention (softmax over the QUERY axis).

reference:
    scores  = einsum('bqd,bkd->bqk', q, k) / sqrt(64)
    weights = softmax(scores, axis=1)          # over q, NOT k!
    out     = einsum('bqk,bkd->bqd', weights, v)

Because the softmax normalizes over q for each (b, k) column, we work with the
transposed score matrix T = K @ Q^T (shape [k, q]): the reduction axis (q) is
then the free axis, which the ACT accum_out reduction handles for free, and the
normalizer Z[k] lives on the contraction axis of the second matmul so it can be
folded into V (V' = V / Z) instead of rescaling the whole [k, q] tile.

Sharding: B=16 batches, data-parallel over 8 cores => 2 batches per core.
The two batches of a core are packed into the two 64-partition halves of
[128, *] tiles ((b, d) packing), which lets pairs of matmuls run concurrently
in disjoint PE-array row strips (scores) / column strips (AV).
"""

import math
from contextlib import ExitStack

import numpy as np

import concourse.bass as bass  # noqa: F401  (kept for symmetry with docs)
import concourse.mybir as mybir
import concourse.tile as tile
from bass_rust import add_dep_helper
from concourse import bacc, bass_utils
from concourse.masks import make_identity

FP32 = mybir.dt.float32
BF16 = mybir.dt.bfloat16
I16 = mybir.dt.int16

N_CORES = 8
B_FULL = 16
BPC = B_FULL // N_CORES  # batches per core = 2
S = 2048
D = 64
NCH = S // 128  # 16 key chunks of 128
SCALE = 1.0 / math.sqrt(D)

# Schraudolph bf16 exp on the DVE: bf16 = bitcast(int16(round(x + B_SCH)))
# approximates exp(s_raw/8) when the psum scores arrive prescaled as
# x = A_SCH * s_raw (A_SCH is folded into the K bf16 cast).  The softmax
# renormalization cancels the approximation's mean error; offloading 5 of the
# 32 (batch, chunk) exps this way relieves the ACT engine, which paces B1.
A_SCH = float(2.0**7 / math.log(2.0) / 8.0)
B_SCH = 16255.0
# (i, b) chunks routed to the DVE exp: all in the i<11 region, whose softmax
# normalizer already comes from the DVE fold tree (exp-source agnostic).
SCHRAUD = frozenset({(2, 0), (4, 1), (6, 0), (8, 1), (10, 0)})


def emit_kernel(ctx: ExitStack, tc, q, k, v, o):
    """Emit the per-core Tile program. q/k/v/o are DRAM APs of [BPC, S, D] f32."""
    nc = tc.nc

    const_pool = ctx.enter_context(tc.tile_pool(name="const", bufs=1))
    big = ctx.enter_context(tc.tile_pool(name="big", bufs=1))
    dram = ctx.enter_context(tc.tile_pool(name="dram", bufs=1, space="DRAM"))
    # PSUM: phase B1 double-buffers two [128,2048] score tiles (all 8 banks);
    # phase B2 reuses the same pool for the [128,2048] O^T accumulator and
    # the [128,128] transpose tiles.
    ps = ctx.enter_context(tc.tile_pool(name="ps", bufs=2, space="PSUM"))

    identb = const_pool.tile([128, 128], BF16, name="identb")
    make_identity(nc, identb)
    zw = const_pool.tile([128, 128], BF16, name="zw")
    nc.vector.memset(zw[:], 0.0)

    # (b,d)-packed transposed operands: partitions 0:64 = batch0 d, 64:128 = batch1 d.
    QT = big.tile([128, S], BF16, name="QT")
    KT = big.tile([128, S], BF16, name="KT")
    # staging for Q/K chunks in (m, b, d) column layout, s on partitions
    qstage = big.tile([128, S], FP32, name="qstage")
    kstage = big.tile([128, S], FP32, name="kstage")
    # V chunks [128 k, 64 d] (f32 as loaded) and Vs = V / Z (bf16)
    V = big.tile([128, BPC * NCH * D], FP32, name="V")
    Vs = big.tile([128, BPC * NCH * D], BF16, name="Vs")
    # per (b, chunk) stats columns: [z, 1/z]
    stats = big.tile([128, BPC * NCH * 2], FP32, name="stats")
    # E[(b*NCH+i)*S :+ S] = exp(scores/sqrt(D)): [128 k, 2048 q] bf16, fully resident
    E = big.tile([128, BPC * NCH * S], BF16, name="E")
    # O^T staging ((b,d) packed on partitions, q on free), bf16 so the
    # output transposes run on the cheap bf16 path (~0.4% output rounding,
    # well inside the 2e-2 gate)
    OT = big.tile([128, S], BF16, name="OT")
    # O in natural layout: column chunk m holds [q-tile m, (b d)]
    O_all = big.tile([128, S], FP32, name="O_all")
    # dummy destination for the DVE-side normalizer reductions
    zscr = big.tile([128, S], BF16, name="zscr")

    # bf16 copies of the staged (s, (b d)) matrices
    qbf = big.tile([128, S], BF16, name="qbf")
    kbf = big.tile([128, S], BF16, name="kbf")
    kbf_dram = dram.tile([S, 128], BF16, name="kbf_dram")

    # ---------------- phase A: load + transpose Q/K, load V ----------------
    # Loads: quarter-granularity strided stage DMAs, batch 0 on the sync
    # HWDGE queue and batch 1 on the scalar queue; V on the otherwise-idle
    # SWDGE (gpsimd) path. Transposes: chunk 0's scores read ALL of QT plus
    # KT chunk 0, so q0..15+k0 go through fast PE transposes (bf16, psum)
    # with the copies on the pre-B1-idle ACT engine; KT chunks 1..15 arrive
    # slightly later via one whole-tensor xbar DMA (bf16 DRAM roundtrip).
    # All stage DMAs ride the sync queue: the scalar queue must stay clear for
    # the transpose copies + ACTIVATEs (a stalled DMA there blocks B1), and q
    # loads fully before k so chunk 0 can start earliest.
    QRT = NCH // 4  # chunks per quarter-DMA
    last_q_dma = None
    # k's first quarter is needed by the transpose chain before q's last two
    # quarters, so interleave it into the load stream
    for src, stg, Q in (
        (q, qstage, 0),
        (q, qstage, 1),
        (k, kstage, 0),
        (q, qstage, 2),
        (q, qstage, 3),
        (k, kstage, 1),
        (k, kstage, 2),
        (k, kstage, 3),
    ):
        ssl = slice(Q * QRT * 128, (Q + 1) * QRT * 128)
        for b in range(BPC):
            # the very first quarter's two batches land in parallel (b1 on the
            # scalar queue, which is idle this early) so the cast/transpose
            # chain starts sooner; everything else stays on sync.
            eng = nc.scalar if (Q == 0 and src is q and b == 1) else nc.sync
            dma = eng.dma_start(
                stg[:, ssl].rearrange("p (m b d) -> p m b d", m=QRT, b=BPC, d=D)[
                    :, :, b, :
                ],
                src[b, ssl, :].rearrange("(m p) d -> p m d", p=128),
            )
            if src is q:
                last_q_dma = dma
    for b in range(BPC):
        vdma = nc.gpsimd.dma_start(
            V[:].rearrange("p (b m d) -> p b m d", b=BPC, m=NCH)[:, b, :, :],
            v[b].rearrange("(m p) d -> p m d", p=128),
        )
        # V isn't needed until well into B1 — keep its HBM traffic out of the
        # way of the critical q loads
        add_dep_helper(
            vdma.ins, last_q_dma.ins, sync=True, reason="delay V behind q loads"
        )
    for Q in range(4):
        csl = slice(Q * QRT * 128, (Q + 1) * QRT * 128)
        nc.vector.tensor_copy(qbf[:, csl], qstage[:, csl])
        # K is prescaled by A_SCH so the DVE schraudolph exp is a single ADD
        nc.vector.tensor_scalar_mul(kbf[:, csl], kstage[:, csl], A_SCH)
    # PE-transposed chunks: q0..15 then k0..3 (the chunks B1 needs before the
    # DRAM-roundtrip xbar below can deliver the rest of KT)
    for idx, (t, m) in enumerate(
        [("q", mm) for mm in range(8)]
        + [("k", mm) for mm in range(4)]
        + [("q", mm) for mm in range(8, NCH)]
    ):
        bft, dst = (qbf, QT) if t == "q" else (kbf, KT)
        pt = ps.tile([128, 128], BF16, tag="ps", name=f"pt_{t}{m}")
        nc.tensor.transpose(pt[:], bft[:, m * 128 : (m + 1) * 128], identb[:])
        # alternate the psum-drain copies between the two idle engines so the
        # copy stage isn't the chain's rate limiter
        if idx % 2 == 0:
            nc.scalar.copy(dst[:, m * 128 : (m + 1) * 128], pt[:])
        else:
            nc.vector.tensor_copy(dst[:, m * 128 : (m + 1) * 128], pt[:])
    # KT chunks 4..15 via DRAM-roundtrip whole-tensor xbar transpose
    nc.sync.dma_start(
        kbf_dram[512:S, :].rearrange("(m p) c -> p m c", p=128),
        kbf[:, 512:S].rearrange("p (m c) -> p m c", m=NCH - 4),
    )
    nc.sync.dma_start_transpose(out=KT[:, 512:S], in_=kbf_dram[512:S, :])

    # ---------------- phase B1: scores -> exp, double-buffered --------------
    # Two [128,2048] score tiles rotate through all 8 PSUM banks; one N=2048
    # exp per (batch, chunk) with accum_out giving the softmax normalizer Z
    # directly (softmax axis == free axis).
    for i in range(NCH):
        for b in range(BPC):
            sct = ps.tile([128, S], FP32, tag="ps", name=f"sc{i}_{b}")
            for j in range(4):
                nc.tensor.matmul(
                    sct[:, j * 512 : (j + 1) * 512],
                    lhsT=KT[b * 64 : (b + 1) * 64, i * 128 : (i + 1) * 128],
                    rhs=QT[b * 64 : (b + 1) * 64, j * 512 : (j + 1) * 512],
                    start=True,
                    stop=True,
                )
            sb = (b * NCH + i) * 2
            eb = (b * NCH + i) * S
            if i < 11:
                # Z for early chunks isn't needed until B2, and the DVE is
                # mostly idle during B1 — reduce E there and spare the ACT
                # queue the 286ns accumulator read.  Fold tree (two 2x-rate
                # bf16 adds + a short reduce, 1.8us) instead of a straight
                # 1x FD=2048 reduce (2.28us).
                if (i, b) in SCHRAUD:
                    # approximate exp on the DVE to relieve the ACT engine
                    nc.vector.tensor_scalar(
                        E[:, eb : eb + S].bitcast(I16),
                        sct[:],
                        B_SCH,
                        None,
                        mybir.AluOpType.add,
                    )
                else:
                    nc.scalar.activation(
                        E[:, eb : eb + S],
                        sct[:],
                        mybir.ActivationFunctionType.Exp,
                        scale=SCALE / A_SCH,
                    )
                nc.vector.tensor_add(
                    zscr[:, 0:1024], E[:, eb : eb + 1024], E[:, eb + 1024 : eb + S]
                )
                nc.vector.tensor_add(
                    zscr[:, 1024:1536], zscr[:, 0:512], zscr[:, 512:1024]
                )
                nc.vector.tensor_reduce(
                    stats[:, sb : sb + 1],
                    zscr[:, 1024:1536],
                    mybir.AxisListType.X,
                    mybir.AluOpType.add,
                )
            else:
                nc.scalar.activation(
                    E[:, eb : eb + S],
                    sct[:],
                    mybir.ActivationFunctionType.Exp,
                    scale=SCALE / A_SCH,
                    accum_out=stats[:, sb : sb + 1],
                )
            vb = (b * NCH + i) * D
            nc.vector.reciprocal(stats[:, sb + 1 : sb + 2], stats[:, sb : sb + 1])
            nc.vector.tensor_scalar_mul(
                Vs[:, vb : vb + D], V[:, vb : vb + D], stats[:, sb + 1 : sb + 2]
            )
    # ---------------- phase B2: dense AV accumulation -----------------------
    # Open the accumulator: each bank gets a full-128-partition zeroing matmul
    # (zero weights) that writes 0 everywhere and sets has_written for the
    # whole bank on every execution, so the partition-sliced AV matmuls can
    # all accumulate with start=False regardless of how the HW scopes the
    # first_mm bank-clear across partitions.
    pot = ps.tile([128, S], FP32, tag="ps", name="pot")
    zmm = []
    for j in range(4):
        zmm.append(
            nc.tensor.matmul(
                pot[:, j * 512 : (j + 1) * 512],
                lhsT=zw[:],
                rhs=QT[:, 0:512],
                start=True,
                stop=False,
                skip_group_check=True,
            )
        )
    o_view = O_all[:].rearrange("p (m b d) -> p m b d", m=NCH, b=BPC, d=D)

    def emit_av(i, j):
        for b in range(BPC):
            # O^T[(b,d), q] += Vs_i^T @ E_i ; b0 -> PE cols 0:63,
            # b1 -> cols 64:127 (concurrent via col tiling)
            vb = (b * NCH + i) * D
            eb = (b * NCH + i) * S
            mm = nc.tensor.matmul(
                pot[b * 64 : (b + 1) * 64, j * 512 : (j + 1) * 512],
                lhsT=Vs[:, vb : vb + D],
                rhs=E[:, eb + j * 512 : eb + (j + 1) * 512],
                start=False,
                stop=(i == NCH - 1 and b == BPC - 1),
                skip_group_check=True,
            )
            if i == 0:
                add_dep_helper(
                    mm.ins,
                    zmm[j].ins,
                    sync=False,
                    reason="AV accumulation after bank-opening zero matmul",
                )

    # dense i-outer accumulation; each region is unpacked (ACT engine) right
    # after its closing matmul so pot's psum slot frees as early as possible —
    # then the 16 out-transposes run with BOTH slots available (2-deep)
    # instead of serializing against the pot-pinned pool.
    for i in range(NCH - 1):
        for j in range(4):
            emit_av(i, j)
    for j in range(4):
        emit_av(NCH - 1, j)
        # alternate the unpacks across ACT and DVE so they drain in parallel
        if j % 2 == 0:
            nc.vector.tensor_copy(
                OT[:, j * 512 : (j + 1) * 512], pot[:, j * 512 : (j + 1) * 512]
            )
        else:
            nc.scalar.copy(
                OT[:, j * 512 : (j + 1) * 512], pot[:, j * 512 : (j + 1) * 512]
            )
    for j in range(4):
        for m in range(4 * j, 4 * j + 4):
            ptc = ps.tile([128, 128], BF16, tag="ps", name=f"ptc_{m}")
            nc.tensor.transpose(ptc[:], OT[:, m * 128 : (m + 1) * 128], identb[:])
            if m % 2 == 0:
                nc.vector.tensor_copy(O_all[:, m * 128 : (m + 1) * 128], ptc[:])
            else:
                nc.scalar.copy(O_all[:, m * 128 : (m + 1) * 128], ptc[:])
        for b in range(BPC):
            # batch 1 outputs ride the idle SWDGE ring so the two batches'
            # stores drain concurrently at the tail
            eng = nc.sync if b == 0 else nc.gpsimd
            eng.dma_start(
                o[b, 4 * j * 128 : (4 * j + 4) * 128, :].rearrange(
                    "(m p) d -> p m d", p=128
                ),
                o_view[:, 4 * j : 4 * j + 4, b, :],
            )


_CACHE: dict = {}


def build_program():
    if "nc" in _CACHE:
        return _CACHE["nc"]
    nc = bacc.Bacc("TRN2", target_bir_lowering=False, debug=False)
    q = nc.dram_tensor("q", [BPC, S, D], FP32, kind="ExternalInput").ap()
    k = nc.dram_tensor("k", [BPC, S, D], FP32, kind="ExternalInput").ap()
    v = nc.dram_tensor("v", [BPC, S, D], FP32, kind="ExternalInput").ap()
    o = nc.dram_tensor("o", [BPC, S, D], FP32, kind="ExternalOutput").ap()
    with tile.TileContext(nc) as tc:
        with ExitStack() as ctx:
            emit_kernel(ctx, tc, q, k, v, o)
    nc.compile()
    _CACHE["nc"] = nc
    return nc


def make_in_maps(q, k, v):
    q = np.ascontiguousarray(q, dtype=np.float32)
    k = np.ascontiguousarray(k, dtype=np.float32)
    v = np.ascontiguousarray(v, dtype=np.float32)
    assert q.shape == (B_FULL, S, D), q.shape
    return [
        {
            "q": np.ascontiguousarray(q[c * BPC : (c + 1) * BPC]),
            "k": np.ascontiguousarray(k[c * BPC : (c + 1) * BPC]),
            "v": np.ascontiguousarray(v[c * BPC : (c + 1) * BPC]),
        }
        for c in range(N_CORES)
    ]


def kernel(q, k, v, _trace=False):
    nc = build_program()
    in_maps = make_in_maps(q, k, v)
    try:
        res = bass_utils.run_bass_kernel_spmd(
            nc, in_maps, core_ids=list(range(N_CORES)), trace=_trace
        )
    except Exception:
        # This box occasionally throws a transient device error
        # (NRT_EXEC_UNIT_UNRECOVERABLE); a short pause + one retry has
        # recovered it every time it was observed.
        import time

        time.sleep(5)
        res = bass_utils.run_bass_kernel_spmd(
            nc, in_maps, core_ids=list(range(N_CORES)), trace=_trace
        )
    out = np.concatenate([r["o"] for r in res.results], axis=0)
    if _trace:
        return out, res
    return out

